# revision 37
# baseline (speedup 1.0000x reference)
"""Trainium2 Bass kernel for nn_GaussianBasis (2D gaussian-splat sum rasterizer).

Math: out[c,d,h,w] = sum_n opacity_n * exp(-sigma_n(h,w)) * features[c,n,d]
where sigma is a per-gaussian quadratic form in pixel coords.

Strategy (v3):
  - Each of the 8 cores owns one 32-row h-band (pixel sharding, no
    collectives). Gaussians are binned host-side into 16x16-px tiles with a
    sigma <= SIG_CUT cutoff ellipse (CUT=8: dropped tails < 1e-5 rel).
  - sigma over a tile is a K=6 matmul: sigma[k,px] = W6[:,k]^T @ phi[:,px],
    phi = [x^2, y^2, x*y, x, y, 1] in tile-CENTERED coords (every entry a
    quarter-integer <= 60.25, exact in fp16); W6 split hi/lo into two fp16
    halves folds into ONE K=12 fp16 matmul.
  - 2-column PACKS: all gaussians of a column pair (4 half-tiles, <= 128
    total) share one 128-slot sigma pass, halving ACT's exp work; slots are
    assigned freely because per-column BLOCK-DIAGONAL feature matrices
    (top-half features in channel cols 0:48, bottom in 48:96, zeros
    elsewhere) route each slot to its channel block in a single K=128
    matmul per column -- the zero off-blocks cancel cross-half mixing.
  - Pipeline per core: PE sigma (8 matmuls) -> ACT exp in (1,2,2,2,1)-pack
    groups -> PE feature matmuls (16) -> fp32->fp16 PSUM->SBUF copies split
    DVE/ACT (ACT takes over once its exps drain; GPSIMD cannot read PSUM)
    -> overlapped [96,*] fp16 DMA flushes on both HWDGE queues, sized so
    the final flush (and its fixed HWDGE+DGE+sem chain) is small.
  - Host reassembles the [96, 4096] fp16 per-band tensors into the
    [C,3,H,W] fp32 output.
"""

import sys
import os

sys.path.insert(0, "/opt/trn_rl_repo")

import numpy as np
from contextlib import ExitStack

N, C, H, W = 2048, 16, 256, 256
NCORES = 8
BH, BW = 32, 32               # bucket (tile) size in pixels
NBH, NBW = H // BH, W // BW   # 8 h-bands (one per core), 8 w-cols
PX = BH * BW                  # 1024 pixels per bucket
CHUNK = 512                   # pixels per matmul (one PSUM bank of fp32 out)
NCH = PX // CHUNK             # 2 chunks per bucket
SIG_CUT = 8.0                 # exp(-8) ~ 3.4e-4: rel err 8e-6 verified vs ref

_cached = {}


def _host_prep(xyz_raw, cholesky_raw, features, opacity):
    """Bin gaussians into (band, col) buckets; emit per-bucket quadratic
    coefficients (bucket-centered coords, fp16 hi/lo split) and
    opacity-folded feature matrices."""
    xy = np.tanh(xyz_raw.astype(np.float64))
    cx = 0.5 * (xy[:, 0] + 1.0) * W
    cy = 0.5 * (xy[:, 1] + 1.0) * H
    chol = cholesky_raw.astype(np.float64) + np.array([0.5, 0.0, 0.5])
    l1, l2, l3 = chol[:, 0], chol[:, 1], chol[:, 2]
    a = l1 * l1
    b = l1 * l2
    c = l2 * l2 + l3 * l3
    det = a * c - b * b
    Aq = 0.5 * (c / det)      # coeff of dx^2
    Bq = -b / det             # coeff of dx*dy
    Cq = 0.5 * (a / det)      # coeff of dy^2
    # ellipse {sigma <= SIG_CUT} axis-aligned bounding half-widths
    rx = np.sqrt(2.0 * SIG_CUT * a) + 1.0
    ry = np.sqrt(2.0 * SIG_CUT * c) + 1.0

    featw = features.astype(np.float64) * opacity[:, 0][None, :, None]  # [C,N,3]
    featw = np.transpose(featw, (1, 0, 2)).reshape(N, C * 3)            # [N,48]

    buckets = [[[] for _ in range(NBW)] for _ in range(NBH)]
    h_lo = np.floor(cy - ry).astype(int)
    h_hi = np.ceil(cy + ry).astype(int)
    w_lo = np.floor(cx - rx).astype(int)
    w_hi = np.ceil(cx + rx).astype(int)
    for n in range(N):
        for bh in range(max(0, h_lo[n] // BH), min(NBH, h_hi[n] // BH + 1)):
            for bw in range(max(0, w_lo[n] // BW), min(NBW, w_hi[n] // BW + 1)):
                buckets[bh][bw].append(n)

    kmax = max(len(buckets[i][j]) for i in range(NBH) for j in range(NBW))
    NT = max(1, (kmax + 127) // 128)
    K_pad = NT * 128

    # Arrays laid out exactly as the SBUF tiles expect, so each input is ONE
    # contiguous DMA: w12 [12, NBW*K_pad], feat [128, NBW*NT*48].
    w12 = np.zeros((NBH, 12, NBW * K_pad), dtype=np.float16)
    feat = np.zeros((NBH, 128, NBW * NT * 48), dtype=np.float16)
    for bh in range(NBH):
        for bw in range(NBW):
            ns = np.array(buckets[bh][bw], dtype=int)
            k = len(ns)
            if k == 0:
                continue
            cxl = cx[ns] - bw * BW - BW / 2
            cyl = cy[ns] - bh * BH - BH / 2
            An, Bn, Cn = Aq[ns], Bq[ns], Cq[ns]
            W6 = np.stack(
                [
                    An,
                    Cn,
                    Bn,
                    -(2.0 * An * cxl + Bn * cyl),
                    -(2.0 * Cn * cyl + Bn * cxl),
                    An * cxl * cxl + Cn * cyl * cyl + Bn * cxl * cyl,
                ],
                0,
            )
            W_hi = W6.astype(np.float16)
            W_lo = (W6 - W_hi.astype(np.float64)).astype(np.float16)
            w12[bh, :6, bw * K_pad:bw * K_pad + k] = W_hi
            w12[bh, 6:, bw * K_pad:bw * K_pad + k] = W_lo
            fk = featw[ns].astype(np.float16)            # [k, 48]
            for nt in range((k + 127) // 128):
                p = min(128, k - nt * 128)
                feat[bh, :p, (bw * NT + nt) * 48:(bw * NT + nt + 1) * 48] = \
                    fk[nt * 128:nt * 128 + p]

    # bucket-centered pixel coords: every entry a quarter-integer <= 240.25,
    # exact in fp16
    xs = (np.arange(BW) + 0.5 - BW / 2).astype(np.float32)
    ys = (np.arange(BH) + 0.5 - BH / 2).astype(np.float32)
    Yg, Xg = np.meshgrid(ys, xs, indexing="ij")
    phi6 = np.stack(
        [Xg * Xg, Yg * Yg, Xg * Yg, Xg, Yg, np.ones_like(Xg)], 0
    ).reshape(6, PX)
    phi12 = np.concatenate([phi6, phi6], 0).astype(np.float16)  # [12, PX]
    return w12, feat, phi12, NT


def _build_program(NT):
    import concourse.bacc as bacc
    import concourse.tile as tile
    import concourse.mybir as mybir

    nc = bacc.Bacc("TRN2", target_bir_lowering=False, debug=False,
                   num_devices=NCORES)
    KP = NT * 128
    w12_ap = nc.dram_tensor("w12", [12, NBW * KP], mybir.dt.float16,
                            kind="ExternalInput").ap()
    feat_ap = nc.dram_tensor("feat", [128, NBW * NT * 48], mybir.dt.float16,
                             kind="ExternalInput").ap()
    phi_ap = nc.dram_tensor("phi", [12, PX], mybir.dt.float16,
                            kind="ExternalInput").ap()
    out_ap = nc.dram_tensor("out", [C * 3, BH, W], mybir.dt.float32,
                            kind="ExternalOutput").ap()

    HB = BH // NCH  # h-rows per chunk (16)
    with tile.TileContext(nc) as tc:
        with ExitStack() as ctx:
            consts = ctx.enter_context(tc.tile_pool(name="consts", bufs=1))
            spool = ctx.enter_context(
                tc.tile_pool(name="sig", bufs=3, space="PSUM"))
            opool = ctx.enter_context(
                tc.tile_pool(name="acc", bufs=2, space="PSUM"))
            gpool = ctx.enter_context(tc.tile_pool(name="g", bufs=3))

            # PE HAM warmup: dummy matmuls on a zeroed SBUF tile while the
            # input DMAs are in flight, so real matmuls start at 2.4 GHz.
            # They rotate through the same psum_s slots as the real sigma
            # matmuls (same tag), serializing only on PE, which is idle.
            dummy = consts.tile([12, 640], mybir.dt.float16)
            nc.vector.memset(dummy, 0)
            for _ in range(2):
                psum_s = spool.tile([128, PX], mybir.dt.float32)
                nc.tensor.matmul(psum_s[:, 0:CHUNK], dummy[:, 0:128],
                                 dummy[:, 128:640], start=True, stop=True)

            # inputs: one contiguous DMA each; phi+w12 on the SP HWDGE queue
            # (ACT's queue is busy with the exp table load), feat on SWDGE
            phi_sb = consts.tile([12, PX], mybir.dt.float16)
            nc.sync.dma_start(out=phi_sb, in_=phi_ap)
            w12_sb = consts.tile([12, NBW * KP], mybir.dt.float16)
            nc.sync.dma_start(out=w12_sb, in_=w12_ap)
            feat_sb = consts.tile([128, NBW * NT * 48], mybir.dt.float16)
            nc.gpsimd.dma_start(out=feat_sb, in_=feat_ap)

            # final band accumulator in SBUF: partitions [0:48] hold chunk 0
            # (h 0..15), [64:112] chunk 1 (h 16..31); free dim is the DRAM
            # band layout (h-major, w global) so the output DMA is contiguous
            out_sb = consts.tile([112, HB * W], mybir.dt.float32)

            for col in range(NBW):
                psum_o = opool.tile([112, CHUNK], mybir.dt.float32)
                for nt in range(NT):
                    psum_s = spool.tile([128, PX], mybir.dt.float32)
                    for ch in range(NCH):
                        nc.tensor.matmul(
                            psum_s[:, ch * CHUNK:(ch + 1) * CHUNK],
                            w12_sb[:, (col * NT + nt) * 128:(col * NT + nt + 1) * 128],
                            phi_sb[:, ch * CHUNK:(ch + 1) * CHUNK],
                            start=True, stop=True)
                    g = gpool.tile([128, PX], mybir.dt.float16)
                    nc.scalar.activation(
                        g, psum_s, mybir.ActivationFunctionType.Exp,
                        bias=0.0, scale=-1.0)
                    for ch in range(NCH):
                        nc.tensor.matmul(
                            psum_o[64 * ch:64 * ch + 48, :],
                            feat_sb[:, (col * NT + nt) * 48:(col * NT + nt + 1) * 48],
                            g[:, ch * CHUNK:(ch + 1) * CHUNK],
                            start=(nt == 0), stop=(nt == NT - 1),
                            tile_position=(0, 64 * ch))
                nc.vector.tensor_copy(
                    out_sb.rearrange("p (h cw) -> p h cw", cw=W)[
                        :, :, col * BW:(col + 1) * BW],
                    psum_o.rearrange("p (h w) -> p h w", w=BW))

            # two contiguous output DMAs: partitions [0:48] -> h rows 0..15,
            # [64:112] -> h rows 16..31
            for ch in range(NCH):
                nc.sync.dma_start(
                    out=out_ap[:, ch * HB:(ch + 1) * HB, :],
                    in_=out_sb[64 * ch:64 * ch + 48, :].rearrange(
                        "p (h cw) -> p h cw", cw=W))
    nc.compile()
    return nc


def _host_prep_packed(cx, cy, Aq, Bq, Cq, rx, ry, featw):
    """16x16-px buckets, two vertical halves packed per 128-partition tile
    (top half-band -> partitions 0:64, bottom -> 64:128). Requires every
    bucket to hold <= 64 gaussians; returns None if not."""
    BH2 = BW2 = 16
    ncol = W // BW2                       # 16 cols per band
    nrow = H // BH2                       # 16 half-band rows
    buckets = [[[] for _ in range(ncol)] for _ in range(nrow)]
    h_lo = np.floor(cy - ry).astype(int)
    h_hi = np.ceil(cy + ry).astype(int)
    w_lo = np.floor(cx - rx).astype(int)
    w_hi = np.ceil(cx + rx).astype(int)
    for n in range(N):
        for bh in range(max(0, h_lo[n] // BH2), min(nrow, h_hi[n] // BH2 + 1)):
            for bw in range(max(0, w_lo[n] // BW2), min(ncol, w_hi[n] // BW2 + 1)):
                buckets[bh][bw].append(n)
    if max(len(buckets[i][j]) for i in range(nrow) for j in range(ncol)) > 64:
        return None

    PX2 = BH2 * BW2
    w12 = np.zeros((NCORES, 12, PX2 + ncol * 128), dtype=np.float16)
    # block-diagonal per column: top gaussians (rows 0:64) carry their 48
    # features in cols 0:48, bottom gaussians (rows 64:128) in cols 48:96 --
    # one K=128 matmul then computes both half-bands' channels with the
    # zero off-blocks cancelling cross-half mixing exactly
    feat = np.zeros((NCORES, 128, ncol * 96), dtype=np.float16)
    for core in range(NCORES):
        for col in range(ncol):
            for half in range(2):
                ns = np.array(buckets[2 * core + half][col], dtype=int)
                k = len(ns)
                if k == 0:
                    continue
                cxl = cx[ns] - col * BW2 - BW2 / 2
                cyl = cy[ns] - (2 * core + half) * BH2 - BH2 / 2
                An, Bn, Cn = Aq[ns], Bq[ns], Cq[ns]
                W6 = np.stack(
                    [
                        An,
                        Cn,
                        Bn,
                        -(2.0 * An * cxl + Bn * cyl),
                        -(2.0 * Cn * cyl + Bn * cxl),
                        An * cxl * cxl + Cn * cyl * cyl + Bn * cxl * cyl,
                    ],
                    0,
                )
                W_hi = W6.astype(np.float16)
                W_lo = (W6 - W_hi.astype(np.float64)).astype(np.float16)
                base = PX2 + col * 128 + 64 * half
                w12[core, :6, base:base + k] = W_hi
                w12[core, 6:, base:base + k] = W_lo
                feat[core, 64 * half:64 * half + k,
                     col * 96 + 48 * half:col * 96 + 48 * half + 48] = \
                    featw[ns].astype(np.float16)

    xs = (np.arange(BW2) + 0.5 - BW2 / 2).astype(np.float32)
    ys = (np.arange(BH2) + 0.5 - BH2 / 2).astype(np.float32)
    Yg, Xg = np.meshgrid(ys, xs, indexing="ij")
    phi6 = np.stack(
        [Xg * Xg, Yg * Yg, Xg * Yg, Xg, Yg, np.ones_like(Xg)], 0
    ).reshape(6, BH2 * BW2)
    phi12 = np.concatenate([phi6, phi6], 0).astype(np.float16)  # [12, 256]
    w12[:, :, 0:PX2] = phi12[None]
    return w12, feat, phi12


def _build_program_v2(spool_bufs=2, opool_bufs=4, gpool_bufs=4,
                      p6_act_copy=True, final_split=False, ahead=2):
    """Grouped sigma/exp + block-diagonal feature matmuls, fp16 staging,
    overlapped flushes.

    Per 4-column group: PE does 4 sigma matmuls (K=12 fp16, F=256 each) into
    psum_s [128,1024]; ACT does ONE exp over the group (PSUM fp32 -> SBUF
    fp16 g). Per column pair: 2 feature matmuls (K=128, block-diagonal
    features -> channels for BOTH half-bands in one pass, out [96,256])
    accumulate in psum_o [128,512]; DVE/Pool alternately convert+copy to the
    fp16 band accumulator; chunk flushes stream out on the SP/ACT HWDGE
    queues, with the last two pairs flushed individually so the final tail
    transfer is small. Host reads rows 0:48 (half 0) / 48:96 (half 1).
    """
    import concourse.bacc as bacc
    import concourse.tile as tile
    import concourse.mybir as mybir

    BH2 = BW2 = 16
    ncol = W // BW2                 # 16 packed tiles per core
    PX2 = BH2 * BW2                 # 256 px per bucket
    npair = ncol // 2               # 8 column pairs

    nc = bacc.Bacc("TRN2", target_bir_lowering=False, debug=False,
                   num_devices=NCORES)
    w12_ap = nc.dram_tensor("w12", [12, PX2 + ncol * 128], mybir.dt.float16,
                            kind="ExternalInput").ap()
    feat_ap = nc.dram_tensor("feat", [128, ncol * 96], mybir.dt.float16,
                             kind="ExternalInput").ap()
    out_ap = nc.dram_tensor("out", [96, npair * 512], mybir.dt.float16,
                            kind="ExternalOutput").ap()

    with tile.TileContext(nc) as tc:
        with ExitStack() as ctx:
            consts = ctx.enter_context(tc.tile_pool(name="consts", bufs=1))
            spool = ctx.enter_context(
                tc.tile_pool(name="sig", bufs=spool_bufs, space="PSUM"))
            opool = ctx.enter_context(
                tc.tile_pool(name="acc", bufs=opool_bufs, space="PSUM"))
            gpool = ctx.enter_context(tc.tile_pool(name="g", bufs=gpool_bufs))

            # PE p-state warmup: tiny matmuls on a Pool-memset tile, issued
            # immediately so the PE ramp clock starts as early as possible.
            dummy = consts.tile([12, 130], mybir.dt.float16)
            nc.vector.memset(dummy, 0)
            for _ in range(2):
                psum_s = spool.tile([128, 1024], mybir.dt.float32)
                nc.tensor.matmul(psum_s[:, 0:2], dummy[:, 0:128],
                                 dummy[:, 128:130], start=True, stop=True)

            # inputs: w12 in ONE small DMA (55KB, 153ns transfer) on the SP
            # HWDGE queue; feat via SWDGE so it doesn't contend
            w12_sb = consts.tile([12, PX2 + ncol * 128], mybir.dt.float16)
            nc.sync.dma_start(out=w12_sb, in_=w12_ap)
            phi_sb = w12_sb[:, 0:PX2]
            feat_sb = consts.tile([128, ncol * 96], mybir.dt.float16)
            nc.scalar.dma_start(out=feat_sb, in_=feat_ap)

            # fp16 band accumulator: partitions [0:48] half 0, [48:96]
            # half 1; free dim pair-major (pair p at [p*512:(p+1)*512])
            out_sb = consts.tile([96, npair * 512], mybir.dt.float16)

            ngrp = ncol // 4
            psums, gs = [None] * ngrp, [None] * ngrp

            def emit_sigma(grp):
                psums[grp] = spool.tile([128, 1024], mybir.dt.float32,
                                        name='psum_s')
                for j in range(4):
                    t = 4 * grp + j
                    nc.tensor.matmul(
                        psums[grp][:, j * PX2:(j + 1) * PX2],
                        w12_sb[:, PX2 + t * 128:PX2 + (t + 1) * 128],
                        phi_sb,
                        start=True, stop=True)

            def emit_body(grp):
                g = gpool.tile([128, 1024], mybir.dt.float16)
                nc.scalar.activation(
                    g, psums[grp], mybir.ActivationFunctionType.Exp,
                    bias=0.0, scale=-1.0)
                for pq in range(2):
                    p = 2 * grp + pq
                    psum_o = opool.tile([96, 512], mybir.dt.float32)
                    for j in range(2):
                        t = 2 * p + j
                        nc.tensor.matmul(
                            psum_o[:, j * PX2:(j + 1) * PX2],
                            feat_sb[:, t * 96:(t + 1) * 96],
                            g[:, (2 * pq + j) * PX2:(2 * pq + j + 1) * PX2],
                            start=True, stop=True)
                    # fp32->fp16 convert + free the PSUM bank; alternate
                    # DVE/Pool mid-stream.  For the LAST two pairs every
                    # engine that has gone idle pitches in: ACT (done with
                    # exps) takes p6 whole, p7 splits per column across
                    # Pool and DVE so the final flush can start asap.
                    # fp32->fp16 convert: GPSIMD cannot read PSUM, and one
                    # DVE cannot keep up with ACT's 519ns/pair pace -- so ACT
                    # (idle once its 4 exps are done at ~7.9us) takes the
                    # p5/p6 copies, DEFERRED so they sit after every exp in
                    # ACT's queue; DVE takes the rest incl. the final p7
                    dst = out_sb[:, p * 512:(p + 1) * 512]
                    if p in (5, 6) and p6_act_copy:
                        deferred.append((p, psum_o, dst))
                        continue
                    nc.vector.tensor_copy(dst, psum_o)
                    if p == 7 and p6_act_copy:
                        # final flush emitted in the tail block, after the
                        # deferred ACT copies that produce pair 6's data
                        continue
                    # overlapped chunk flushes on the SP queue as quads
                    # finish; ONE final flush for pairs 6+7 on the ACT queue
                    # (its SEQ is free after the exps, so the wait overlaps
                    # the copies and the chain fires the moment they land)
                    if p == 6 and final_split:
                        nc.sync.dma_start(
                            out=out_ap[:, 3072:3584],
                            in_=out_sb[:, 3072:3584])
                    elif p == 7 and final_split:
                        nc.scalar.dma_start(
                            out=out_ap[:, 3584:4096],
                            in_=out_sb[:, 3584:4096])
                    elif p == 7:
                        nc.scalar.dma_start(
                            out=out_ap[:, 3072:4096],
                            in_=out_sb[:, 3072:4096])
                    elif p % 2 == 1 and p < 6:
                        q = p // 2
                        nc.sync.dma_start(
                            out=out_ap[:, q * 1024:(q + 1) * 1024],
                            in_=out_sb[:, q * 1024:(q + 1) * 1024])

            # software-pipelined emission: sigma for group g+1 goes BEFORE
            # the exp/features of group g so the PE queue keeps ACT fed
            deferred = []
            for gpre in range(min(ahead, ngrp)):
                emit_sigma(gpre)
            for grp in range(ngrp):
                if grp + ahead < ngrp:
                    emit_sigma(grp + ahead)
                emit_body(grp)
            for p, psum_o, dst in deferred:
                nc.scalar.copy(dst, psum_o)
                if p % 2 == 1 and p < 6:
                    q = p // 2
                    nc.sync.dma_start(
                        out=out_ap[:, q * 1024:(q + 1) * 1024],
                        in_=out_sb[:, q * 1024:(q + 1) * 1024])
            if deferred:
                nc.scalar.dma_start(
                    out=out_ap[:, 3072:4096],
                    in_=out_sb[:, 3072:4096])
    nc.compile()
    return nc


def _host_prep_pack(cx, cy, Aq, Bq, Cq, rx, ry, featw):
    """2-column packs: all gaussians of a column PAIR (4 16x16 half-tiles)
    share one 128-slot sigma pass; slots are assigned freely (the per-column
    block-diagonal feature matrices route each slot to its channel block).
    Requires <= 128 gaussians per pair; returns None if exceeded."""
    BT = 16
    buckets = [[[] for _ in range(16)] for _ in range(16)]
    h_lo = np.floor(cy - ry).astype(int)
    h_hi = np.ceil(cy + ry).astype(int)
    w_lo = np.floor(cx - rx).astype(int)
    w_hi = np.ceil(cx + rx).astype(int)
    for n in range(N):
        for bh in range(max(0, h_lo[n] // BT), min(16, h_hi[n] // BT + 1)):
            for bw in range(max(0, w_lo[n] // BT), min(16, w_hi[n] // BT + 1)):
                buckets[bh][bw].append(n)

    PX2 = BT * BT
    w12 = np.zeros((NCORES, 12, PX2 + 8 * 128), dtype=np.float16)
    feat = np.zeros((NCORES, 128, 16 * 96), dtype=np.float16)
    for band in range(NCORES):
        for p in range(8):
            slots = []
            for j in range(2):
                for half in range(2):
                    col = 2 * p + j
                    for n in buckets[2 * band + half][col]:
                        slots.append((n, j, half))
            if len(slots) > 128:
                return None
            for s, (n, j, half) in enumerate(slots):
                col = 2 * p + j
                cxl = cx[n] - col * BT - BT / 2
                cyl = cy[n] - (2 * band + half) * BT - BT / 2
                An, Bn, Cn = Aq[n], Bq[n], Cq[n]
                W6 = np.array([
                    An, Cn, Bn,
                    -(2.0 * An * cxl + Bn * cyl),
                    -(2.0 * Cn * cyl + Bn * cxl),
                    An * cxl * cxl + Cn * cyl * cyl + Bn * cxl * cyl,
                ])
                W_hi = W6.astype(np.float16)
                W_lo = (W6 - W_hi.astype(np.float64)).astype(np.float16)
                base = PX2 + p * 128 + s
                w12[band, :6, base] = W_hi
                w12[band, 6:, base] = W_lo
                feat[band, s, col * 96 + 48 * half:col * 96 + 48 * half + 48] = \
                    featw[n].astype(np.float16)

    xs = (np.arange(BT) + 0.5 - BT / 2).astype(np.float32)
    ys = (np.arange(BT) + 0.5 - BT / 2).astype(np.float32)
    Yg, Xg = np.meshgrid(ys, xs, indexing="ij")
    phi6 = np.stack(
        [Xg * Xg, Yg * Yg, Xg * Yg, Xg, Yg, np.ones_like(Xg)], 0
    ).reshape(6, PX2)
    phi12 = np.concatenate([phi6, phi6], 0).astype(np.float16)
    w12[:, :, 0:PX2] = phi12[None]
    return w12, feat


def _build_program_v3(spool_bufs=3, opool_bufs=5, gpool_bufs=4,
                      act_copies=(3, 5, 6), groups=None):
    """2-column packs: 8 sigma matmuls (one per pack), 4 exp instrs over
    [128,512] groups (half the ACT work of v2), 16 block-diagonal feature
    matmuls; copies split DVE/ACT; overlapped flushes as in v2."""
    import concourse.bacc as bacc
    import concourse.tile as tile
    import concourse.mybir as mybir

    PX2 = 256
    npair = 8

    nc = bacc.Bacc("TRN2", target_bir_lowering=False, debug=False,
                   num_devices=NCORES)
    w12_ap = nc.dram_tensor("w12", [12, PX2 + 8 * 128], mybir.dt.float16,
                            kind="ExternalInput").ap()
    feat_ap = nc.dram_tensor("feat", [128, 16 * 96], mybir.dt.float16,
                             kind="ExternalInput").ap()
    out_ap = nc.dram_tensor("out", [96, npair * 512], mybir.dt.float16,
                            kind="ExternalOutput").ap()

    with tile.TileContext(nc) as tc:
        with ExitStack() as ctx:
            consts = ctx.enter_context(tc.tile_pool(name="consts", bufs=1))
            spool = ctx.enter_context(
                tc.tile_pool(name="sig", bufs=spool_bufs, space="PSUM"))
            opool = ctx.enter_context(
                tc.tile_pool(name="acc", bufs=opool_bufs, space="PSUM"))
            gpool = ctx.enter_context(tc.tile_pool(name="g", bufs=gpool_bufs))

            dummy = consts.tile([12, 130], mybir.dt.float16)
            nc.vector.memset(dummy, 0)
            for _ in range(2):
                psum_s = spool.tile([128, 512], mybir.dt.float32)
                nc.tensor.matmul(psum_s[:, 0:2], dummy[:, 0:128],
                                 dummy[:, 128:130], start=True, stop=True)

            # w12 (one small DMA) on SP; feat in two chunks so the first
            # half's features land before pair 0 needs them
            w12_sb = consts.tile([12, PX2 + 8 * 128], mybir.dt.float16)
            nc.sync.dma_start(out=w12_sb, in_=w12_ap)
            phi_sb = w12_sb[:, 0:PX2]
            feat_sb = consts.tile([128, 16 * 96], mybir.dt.float16)
            nc.scalar.dma_start(out=feat_sb[:, 0:768], in_=feat_ap[:, 0:768])
            nc.sync.dma_start(out=feat_sb[:, 768:], in_=feat_ap[:, 768:])

            out_sb = consts.tile([96, npair * 512], mybir.dt.float16)

            # asymmetric exp groups (in packs): the 1-pack first group lets
            # the exp/feature/copy stream start one sigma earlier
            GROUPS = groups or [(0, 1), (1, 2), (3, 2), (5, 2), (7, 1)]
            psums = [None] * len(GROUPS)

            def emit_sigma(grp):
                start, npk = GROUPS[grp]
                psums[grp] = spool.tile([128, npk * PX2], mybir.dt.float32,
                                        name='psum_s')
                for j in range(npk):
                    pk = start + j
                    nc.tensor.matmul(
                        psums[grp][:, j * PX2:(j + 1) * PX2],
                        w12_sb[:, PX2 + pk * 128:PX2 + (pk + 1) * 128],
                        phi_sb,
                        start=True, stop=True)

            deferred = []

            def emit_body(grp):
                start, npk = GROUPS[grp]
                g = gpool.tile([128, npk * PX2], mybir.dt.float16)
                nc.scalar.activation(
                    g, psums[grp], mybir.ActivationFunctionType.Exp,
                    bias=0.0, scale=-1.0)
                for pq in range(npk):
                    p = start + pq
                    psum_o = opool.tile([96, 512], mybir.dt.float32)
                    for j in range(2):
                        t = 2 * p + j
                        nc.tensor.matmul(
                            psum_o[:, j * PX2:(j + 1) * PX2],
                            feat_sb[:, t * 96:(t + 1) * 96],
                            g[:, pq * PX2:(pq + 1) * PX2],
                            start=True, stop=True)
                    dst = out_sb[:, p * 512:(p + 1) * 512]
                    if p in act_copies:
                        deferred.append((p, psum_o, dst))
                        continue
                    nc.vector.tensor_copy(dst, psum_o)
                    if p == 7:
                        continue          # final flush goes in the tail block
                    if p % 2 == 1 and (p - 1) not in act_copies \
                            and p not in act_copies:
                        q = p // 2
                        nc.sync.dma_start(
                            out=out_ap[:, q * 1024:(q + 1) * 1024],
                            in_=out_sb[:, q * 1024:(q + 1) * 1024])

            emit_sigma(0)
            emit_sigma(1)
            for grp in range(len(GROUPS)):
                if grp + 2 < len(GROUPS):
                    emit_sigma(grp + 2)
                emit_body(grp)
            # tail: deferred ACT copies (sit after every exp in ACT's queue),
            # any flush whose data they produce, then the final flush
            for p, psum_o, dst in deferred:
                nc.scalar.copy(dst, psum_o)
                if p % 2 == 1 and p < 6:
                    q = p // 2
                    nc.sync.dma_start(
                        out=out_ap[:, q * 1024:(q + 1) * 1024],
                        in_=out_sb[:, q * 1024:(q + 1) * 1024])
            nc.scalar.dma_start(
                out=out_ap[:, 3072:4096],
                in_=out_sb[:, 3072:4096])
    nc.compile()
    return nc


def _build_program_packed():
    import concourse.bacc as bacc
    import concourse.tile as tile
    import concourse.mybir as mybir

    BH2 = BW2 = 16
    ncol = W // BW2                 # 16 packed tiles per core
    PX2 = BH2 * BW2                 # 256 px per bucket
    npair = ncol // 2               # col pairs sharing one PSUM/ACT group

    nc = bacc.Bacc("TRN2", target_bir_lowering=False, debug=False,
                   num_devices=NCORES)
    # phi rides in the same tensor as w12 (FIRST PX2 columns), so the first
    # DMA chunk (phi + first 4 col tiles) lands before the rest
    w12_ap = nc.dram_tensor("w12", [12, PX2 + ncol * 128], mybir.dt.float16,
                            kind="ExternalInput").ap()
    feat_ap = nc.dram_tensor("feat", [128, ncol * 48], mybir.dt.float16,
                             kind="ExternalInput").ap()
    out_ap = nc.dram_tensor("out", [C * 3, BH, W], mybir.dt.float32,
                            kind="ExternalOutput").ap()

    with tile.TileContext(nc) as tc:
        with ExitStack() as ctx:
            consts = ctx.enter_context(tc.tile_pool(name="consts", bufs=1))
            spool = ctx.enter_context(
                tc.tile_pool(name="sig", bufs=2, space="PSUM"))
            opool = ctx.enter_context(
                tc.tile_pool(name="acc", bufs=3, space="PSUM"))
            gpool = ctx.enter_context(tc.tile_pool(name="g", bufs=3))

            dummy = consts.tile([12, 640], mybir.dt.float16)
            nc.vector.memset(dummy, 0)
            for _ in range(2):
                psum_s = spool.tile([128, 4 * PX2], mybir.dt.float32)
                nc.tensor.matmul(psum_s[:, 0:512], dummy[:, 0:128],
                                 dummy[:, 128:640], start=True, stop=True)

            w12_sb = consts.tile([12, PX2 + ncol * 128], mybir.dt.float16)
            CUT = PX2 + 4 * 128
            nc.sync.dma_start(out=w12_sb[:, :CUT], in_=w12_ap[:, :CUT])
            nc.sync.dma_start(out=w12_sb[:, CUT:], in_=w12_ap[:, CUT:])
            phi_sb = w12_sb[:, 0:PX2]
            feat_sb = consts.tile([128, ncol * 48 + 8], mybir.dt.float16)
            nc.gpsimd.dma_start(out=feat_sb, in_=feat_ap)
            idxs_sb = feat_sb[0:16, ncol * 48:ncol * 48 + 8].bitcast(
                mybir.dt.int16)

            # band accumulator, h-major DRAM layout; partitions [0:48] hold
            # h 0..15, [64:112] h 16..31
            out_sb = consts.tile([112, (BH // 2) * W], mybir.dt.float32)
            out_v = out_sb.rearrange("p (h cw) -> p h cw", cw=W)

            for qr in range(npair // 2):
                # one 4-col sigma/exp group (fewer ACT instruction overheads)
                psum_s = spool.tile([128, 4 * PX2], mybir.dt.float32)
                for j in range(4):
                    t = 4 * qr + j
                    nc.tensor.matmul(
                        psum_s[:, j * PX2:(j + 1) * PX2],
                        w12_sb[:, PX2 + t * 128:PX2 + (t + 1) * 128],
                        phi_sb,
                        start=True, stop=True)
                g = gpool.tile([128, 4 * PX2], mybir.dt.float16)
                nc.scalar.activation(
                    g, psum_s, mybir.ActivationFunctionType.Exp,
                    bias=0.0, scale=-1.0)
                for pq in range(2):
                    pr = 2 * qr + pq
                    psum_o = opool.tile([96, 512], mybir.dt.float32)
                    for j in range(2):
                        t = 2 * pr + j
                        gj = 2 * pq + j
                        for half in range(2):
                            nc.tensor.matmul(
                                psum_o[64 * half:64 * half + 48,
                                       j * PX2:(j + 1) * PX2],
                                feat_sb[64 * half:64 * half + 64,
                                        t * 48:(t + 1) * 48],
                                g[64 * half:64 * half + 64,
                                  gj * PX2:(gj + 1) * PX2],
                                start=True, stop=True,
                                tile_position=(64 * half, 64 * half))
                    # psum free order (c2, h16, w16) -> out (h-major, global w)
                    nc.vector.tensor_copy(
                        out_v[:, :, pr * 2 * BW2:(pr + 1) * 2 * BW2].rearrange(
                            "p h (c w) -> p c h w", w=BW2),
                        psum_o.rearrange("p (c h w) -> p c h w",
                                         h=BH2, w=BW2))

            for ch in range(2):
                nc.sync.dma_start(
                    out=out_ap[:, ch * (BH // 2):(ch + 1) * (BH // 2), :],
                    in_=out_sb[64 * ch:64 * ch + 48, :].rearrange(
                        "p (h cw) -> p h cw", cw=W))
    nc.compile()
    return nc


def _params(np_inputs):
    """Per-gaussian params (fp64 host): centers, quadratic coeffs, cutoff
    radii, opacity-folded features."""
    xyz_raw = np.asarray(np_inputs["xyz_raw"], dtype=np.float32)
    cholesky_raw = np.asarray(np_inputs["cholesky_raw"], dtype=np.float32)
    features = np.asarray(np_inputs["features"], dtype=np.float32)
    opacity = np.asarray(np_inputs["opacity"], dtype=np.float32)
    xy = np.tanh(xyz_raw.astype(np.float64))
    cx = 0.5 * (xy[:, 0] + 1.0) * W
    cy = 0.5 * (xy[:, 1] + 1.0) * H
    chol = cholesky_raw.astype(np.float64) + np.array([0.5, 0.0, 0.5])
    l1, l2, l3 = chol[:, 0], chol[:, 1], chol[:, 2]
    a = l1 * l1
    b = l1 * l2
    c = l2 * l2 + l3 * l3
    det = a * c - b * b
    Aq, Bq, Cq = 0.5 * (c / det), -b / det, 0.5 * (a / det)
    rx = np.sqrt(2.0 * SIG_CUT * a) + 1.0
    ry = np.sqrt(2.0 * SIG_CUT * c) + 1.0
    featw = features.astype(np.float64) * opacity[:, 0][None, :, None]
    featw = np.transpose(featw, (1, 0, 2)).reshape(N, C * 3)
    return cx, cy, Aq, Bq, Cq, rx, ry, featw


def kernel(xyz_raw, cholesky_raw, features, opacity):
    from concourse.bass_utils import run_bass_kernel_spmd

    xyz_raw = np.asarray(xyz_raw, dtype=np.float32)
    cholesky_raw = np.asarray(cholesky_raw, dtype=np.float32)
    features = np.asarray(features, dtype=np.float32)
    opacity = np.asarray(opacity, dtype=np.float32)

    cx, cy, Aq, Bq, Cq, rx, ry, featw = _params({
        "xyz_raw": xyz_raw, "cholesky_raw": cholesky_raw,
        "features": features, "opacity": opacity})

    pack = _host_prep_pack(cx, cy, Aq, Bq, Cq, rx, ry, featw)
    if pack is not None:
        w12, feat = pack
        if "v3" not in _cached:
            _cached["v3"] = _build_program_v3()
        nc = _cached["v3"]
        in_maps = [
            {"w12": w12[band], "feat": feat[band]} for band in range(NCORES)
        ]
        res = run_bass_kernel_spmd(nc, in_maps, core_ids=list(range(NCORES)))
        out = np.empty((C * 3, H, W), dtype=np.float32)
        for band in range(NCORES):
            # [96, 8*512] fp16: rows 0:48 half 0, 48:96 half 1;
            # free dim: px = p*512 + j*256 + py*16 + pxw
            a = np.asarray(res.results[band]["out"], dtype=np.float32)
            a = a.reshape(2, 48, 8, 2, 16, 16)          # h c p j py pxw
            a = np.transpose(a, (1, 0, 4, 2, 3, 5))     # c h py p j pxw
            out[:, band * BH:(band + 1) * BH, :] = a.reshape(48, 32, 256)
        return out.reshape(C, 3, H, W)

    packed = _host_prep_packed(cx, cy, Aq, Bq, Cq, rx, ry, featw)
    if packed is not None:
        w12, feat, _ = packed
        if "v2" not in _cached:
            _cached["v2"] = _build_program_v2()
        nc = _cached["v2"]
        in_maps = [
            {"w12": w12[band], "feat": feat[band]} for band in range(NCORES)
        ]
        res = run_bass_kernel_spmd(nc, in_maps, core_ids=list(range(NCORES)))
        out = np.empty((C * 3, H, W), dtype=np.float32)
        for band in range(NCORES):
            # [128, 8*512] fp16: rows 0:48 half 0, 48:96 half 1 (rest junk);
            # free dim: px = p*512 + j*256 + py*16 + pxw
            a = np.asarray(res.results[band]["out"][:96], dtype=np.float32)
            a = a.reshape(2, 48, 8, 2, 16, 16)          # h c p j py pxw
            a = np.transpose(a, (1, 0, 4, 2, 3, 5))     # c h py p j pxw
            out[:, band * BH:(band + 1) * BH, :] = a.reshape(48, 32, 256)
        return out.reshape(C, 3, H, W)
    else:
        w12, feat, phi12, NT = _host_prep(
            xyz_raw, cholesky_raw, features, opacity)
        if NT not in _cached:
            _cached[NT] = _build_program(NT)
        nc = _cached[NT]
        in_maps = [
            {"w12": w12[band], "feat": feat[band], "phi": phi12}
            for band in range(NCORES)
        ]
    res = run_bass_kernel_spmd(nc, in_maps, core_ids=list(range(NCORES)))

    out = np.empty((C * 3, H, W), dtype=np.float32)
    for band in range(NCORES):
        out[:, band * BH:(band + 1) * BH, :] = res.results[band]["out"]
    return out.reshape(C, 3, H, W)

